# revision 1
# baseline (speedup 1.0000x reference)
"""Trainium2 Bass kernel for CodeAttention (B=4, S=2048, E=768, H=12).

Sharding: 8 cores = 4 batches x 2 head-groups (6 heads each).
Each core computes a partial projection output for its batch; the host
sums the two partials per batch and adds the (host-folded) bias row.

fp16 datapath, single fused pipeline (~270us/core cost-model estimate,
vs 318us for the f32r phase-split baseline; max rel err ~8.5e-4):
- The padding mask is folded multiplicatively into the V store (masked
  keys get v=0 AND ones-column=0), so exp needs no per-key-chunk bias
  and masked keys drop out of the softmax denominator exactly.
- The attention kc loop is software-pipelined (depth 2): scores(i) are
  emitted before pv(i-2) so the scalar engine's exp stream never waits.
- x-transpose / QKV projection / output projection work is emitted as
  "filler units" inside the ACT-paced attention windows, keeping PE
  busy during exp. PSUM: 3x2-bank score/unit ring + 2-bank pv slot.
- The pv accumulator is evacuated to SBUF fp16 right after each head
  pair so its single PSUM slot recycles; normalization (reciprocal of
  the ones-row, gpsimd partition-broadcast, multiply) runs off PE.
"""

import sys

if "/opt/trn_rl_repo" not in sys.path:
    sys.path.insert(0, "/opt/trn_rl_repo")

import numpy as np

import concourse.bass as bass  # noqa: F401  (engine types referenced via nc)
import concourse.mybir as mybir
import concourse.tile as tile
from concourse import bacc
from concourse.alu_op_type import AluOpType
from concourse.bass_utils import run_bass_kernel_spmd
from concourse.masks import make_identity

F32 = mybir.dt.float32
F32R = mybir.dt.float32r
FP16 = mybir.dt.float16
Act = mybir.ActivationFunctionType

B, S, E, H, D = 4, 2048, 768, 12, 64
HC = 6                    # heads per core
QKC = HC * D * 2          # qk columns per core = 768
VC = HC * D               # v columns per core = 384
KCH = E // 128            # contraction chunks over E = 6
NKC = S // 128            # key chunks = 16
NQB = S // 512            # q blocks of 512 = 4
NSB = S // 512            # s blocks of 512 = 4
VW = D + 1                # v width incl. mask column = 65
NIT = NQB * (HC // 2) * NKC  # flattened attention iterations = 192


def build_program():
    nc = bacc.Bacc("TRN2", target_bir_lowering=False, debug=False, num_devices=8)

    x_d = nc.dram_tensor("x", [S, E], FP16, kind="ExternalInput")
    wqk_d = nc.dram_tensor("wqk", [QKC // 128, KCH, 128, 128], FP16, kind="ExternalInput")
    wv_d = nc.dram_tensor("wv", [KCH, 128, VC], FP16, kind="ExternalInput")
    wp_d = nc.dram_tensor("wp", [VC // 128, 128, E], FP16, kind="ExternalInput")
    bqk_d = nc.dram_tensor("bqk", [QKC], F32, kind="ExternalInput")
    mb_d = nc.dram_tensor("mb", [S], F32, kind="ExternalInput")
    mrep_d = nc.dram_tensor("mrep", [NKC * HC * 128], FP16, kind="ExternalInput")
    y_d = nc.dram_tensor("y", [S, E], F32, kind="ExternalOutput")

    with tile.TileContext(nc) as tc:
        _emit(nc, tc, x_d, wqk_d, wv_d, wp_d, bqk_d, mb_d, mrep_d, y_d)
    nc.compile()
    return nc


def _build_schedule():
    """fillers[i] = list of units to emit inside attention iteration i.

    Units: ("v", sb, sc) v-projection for s-chunk, ("qk", sb, m) qk
    projection m-chunk, ("norm", qb, hp) softmax normalization,
    ("proj", qb, sc) output projection chunk.  Lead-2 scheduling: a unit
    lands >=2 iterations before its first consumer so DVE evacuation of
    the unit's PSUM tile is off PE's critical path.
    """
    fillers = [[] for _ in range(NIT)]

    def it(qb, hp, kc):
        return (qb * (HC // 2) + hp) * NKC + kc

    # transposes for s-chunks 4..15 (sb0 is in the prologue), lead ~6
    # iterations before their first consumer (v unit / qk unit).
    for sg in range(4, NKC):
        fillers[max(0, sg - 6)].append(("tr", sg))
    # v units: consumer pv(qb0,hp0,kc) emitted at iteration kc+1.
    # (sb0,sc0) is in the prologue; the rest land at iteration kc-1.
    for kc in range(1, NKC):
        fillers[max(0, kc - 1)].append(("v", kc // 4, kc % 4))
    # k-part qk units for hp0 (m=3): scores(qb0,hp0,kc=4j) at iteration 4j.
    for j in range(1, 4):
        fillers[4 * j - 2].append(("qk", j, 3))
    # q-parts for qb0 hp1/hp2 (m=1,2 of sb0) + k-parts m=4,5:
    # scores(qb0,hp1,kc) at iteration 16+kc needs qkT[kc//4][m=4];
    # scores(qb0,hp1,*) needs qkT[0][m=1] at iteration 16.
    fillers[13].append(("qk", 0, 1))
    fillers[14].append(("qk", 0, 4))
    for j in range(1, 4):
        fillers[16 + 4 * j - 3].append(("qk", j, 4))
    fillers[16 + 9].append(("qk", 0, 2))
    fillers[16 + 12].append(("qk", 0, 5))
    for j in range(1, 4):
        fillers[32 + 4 * j - 3].append(("qk", j, 5))
    # q-part for qb1-hp0 (needed at iteration 48)
    fillers[32 + 13].append(("qk", 1, 0))
    # q-parts for later q-blocks: qkT[qb][m] needed at it(qb,hp=m,0).
    fillers[it(1, 0, 13)].append(("qk", 1, 1))
    fillers[it(1, 1, 13)].append(("qk", 1, 2))
    fillers[it(1, 2, 5)].append(("qk", 2, 0))
    fillers[it(2, 0, 5)].append(("qk", 2, 1))
    fillers[it(2, 1, 5)].append(("qk", 2, 2))
    fillers[it(2, 2, 5)].append(("qk", 3, 0))
    fillers[it(3, 0, 5)].append(("qk", 3, 1))
    fillers[it(3, 1, 5)].append(("qk", 3, 2))
    # norms: norm(qb,hp) right after pv(qb,hp,15) (emitted at +2 with the
    # depth-2 pv pipeline) so the single pv slot frees quickly.
    for qb in range(NQB):
        for hp in range(HC // 2):
            i = it(qb, hp, NKC - 1) + 3
            if i < NIT:
                fillers[i].insert(0, ("norm", qb, hp))
    # output projection of qb spread through qb+1 (norms released by then),
    # weighted toward the otherwise filler-light hp2 windows.
    for qb in range(NQB - 1):
        for sc, (hp, kc) in enumerate(((0, 6), (1, 4), (2, 2), (2, 8))):
            fillers[it(qb + 1, hp, kc)].append(("proj", qb, sc))
    return fillers


def _emit(nc, tc, x_d, wqk_d, wv_d, wp_d, bqk_d, mb_d, mrep_d, y_d):
    ctx_pools = []

    def pool(name, bufs, space="SBUF"):
        p = tc.tile_pool(name=name, bufs=bufs, space=space)
        ctx_pools.append(p)
        return p.__enter__()

    consts = pool("consts", 1)
    store = pool("store", 1)

    ident = consts.tile([128, 128], FP16)
    ones_row_f = consts.tile([1, D], F32)
    nc.vector.memset(ones_row_f[:], 1.0)
    ones_row = consts.tile([1, D], F32R)
    nc.vector.tensor_copy(ones_row[:], ones_row_f[:])

    # weights go over the SWDGE (gpsimd) queue so the x-chunk loads on the
    # sync HWDGE queue aren't serialized behind the weight traffic.
    # wqk arrives per m-chunk, ordered by first use (m0/m3 feed the first
    # attention iterations). The m0 descriptor-gen goes first on the Pool
    # engine; the identity build slots in after it (the first transpose
    # can't start before the first x chunk lands anyway).
    wqk = consts.tile([128, KCH, QKC], FP16)
    wv = consts.tile([128, KCH, VC], FP16)
    wp = consts.tile([128, VC // 128, E], FP16)

    def load_wqk(m):
        nc.gpsimd.dma_start(
            wqk[:, :, m * 128 : (m + 1) * 128],
            wqk_d.ap()[m].rearrange("k p f -> p k f"),
        )

    load_wqk(0)
    make_identity(nc, ident[:])
    load_wqk(3)
    nc.gpsimd.dma_start(wv[:], wv_d.ap().rearrange("k p f -> p k f"))
    load_wqk(1)
    load_wqk(4)
    load_wqk(2)
    load_wqk(5)
    nc.gpsimd.dma_start(wp[:], wp_d.ap().rearrange("t p f -> p t f"))

    bqk = consts.tile([128, QKC // 128], F32)
    mb = consts.tile([128, NKC], F32)
    mrep = consts.tile([128, NKC, HC], FP16)

    # qkT store: tile m of 6 holds W-columns m*128..; q cols 0..383 (m 0..2),
    # k cols 384..767 (m 3..5).
    qkT = [
        store.tile([128, QKC // 128, 512], FP16, name=f"qkT{sb}")
        for sb in range(NSB)
    ]
    # v store: per s-block [s-chunk, head, 65] with the mask value (0/1) in
    # column 64 — multiplicative padding mask (masked keys contribute 0 to
    # both the numerator and the softmax denominator).
    vst = [
        store.tile([128, 4, HC, VW], FP16, name=f"vst{sb}") for sb in range(NSB)
    ]
    # attn output (transposed): tile t rows = head dims 2t,2t+1.
    att = store.tile([128, VC // 128, S], FP16)
    # x transposed, kept for the whole run (deferred qk/v projections).
    xts = [store.tile([128, KCH, 512], FP16, name=f"xt{sb}") for sb in range(NSB)]

    # ---- Fused pipeline pools ----
    # PSUM: st ring 3x2 banks (scores + transient unit/transpose tiles) +
    # one 2-bank pv accumulator = exactly 8 banks.
    xs_p = pool("xs", NKC)
    st_p = pool("st", 3, space="PSUM")   # [128,1024] f32 = 2 banks each
    pv_p = pool("pv", 1, space="PSUM")   # [128,1024] f32 = 2 banks
    pt_p = pool("pt", 5)
    pf_p = pool("pf", 2)
    rs_p = pool("rs", 2)
    bc_p = pool("bc", 2)
    ys_p = pool("ys", 2)

    # stage all 16 x-chunk loads up front (one SBUF tile each). The small
    # per-partition tables (contiguous per-partition runs, pre-transposed on
    # the host) slot in after the first few chunks: late enough not to delay
    # the first transposes, early enough for the first qk/v evacuations.
    xss = []
    for sg in range(NKC):
        xs = xs_p.tile([128, E], FP16, tag="xs", name=f"xs{sg}")
        nc.sync.dma_start(xs[:], x_d.ap()[sg * 128 : (sg + 1) * 128, :])
        xss.append(xs)
        if sg == 3:
            nc.scalar.dma_start(
                bqk[:], bqk_d.ap().rearrange("(p c) -> p c", p=128)
            )
            nc.scalar.dma_start(
                mb[:], mb_d.ap().rearrange("(p c) -> p c", p=128)
            )
            # mask replicated per head: mrep[p, kc, h] = mask[kc*128 + p]
            nc.scalar.dma_start(
                mrep[:], mrep_d.ap().rearrange("(p c h) -> p c h", p=128, h=HC)
            )

    def unit_tr(sg):
        sb, sc = sg // 4, sg % 4
        tp = st_p.tile([128, 1024], FP16, tag="st", name=f"tp{sg}")
        for k in range(KCH):
            nc.tensor.matmul(
                tp[:, k * 128 : (k + 1) * 128],
                xss[sg][:, k * 128 : (k + 1) * 128], ident[:],
                is_transpose=True,
                start=(k == 0), stop=(k == KCH - 1),
            )
        nc.vector.tensor_copy(
            xts[sb][:, :, sc * 128 : (sc + 1) * 128],
            tp[:, : KCH * 128].rearrange("p (k f) -> p k f", k=KCH),
        )

    def unit_v(sb, sc):
        sg = sb * 4 + sc
        u = st_p.tile([128, 1024], F32, tag="st", name=f"va{sg}")
        for k in range(KCH):
            nc.tensor.matmul(
                u[:, 0:VC], xts[sb][:, k, sc * 128 : (sc + 1) * 128],
                wv[:, k, :],
                start=(k == 0), stop=(k == KCH - 1),
            )
        # multiplicative padding mask folded into the V store (the mask
        # value for key row p is a per-partition scalar here).
        nc.vector.tensor_scalar_mul(
            vst[sb][:, sc, :, 0:D],
            u[:, 0:VC].rearrange("p (h d) -> p h d", h=HC),
            mb[:, sg : sg + 1],
        )
        nc.vector.tensor_copy(
            vst[sb][:, sc, :, D : D + 1],
            mrep[:, sg : sg + 1, :].rearrange("p one b -> p b one"),
        )

    def unit_qk(sb, m):
        u = st_p.tile([128, 1024], F32, tag="st", name=f"qk{sb}_{m}")
        for k in range(KCH):
            nc.tensor.matmul(
                u[:, 0:512], wqk[:, k, m * 128 : (m + 1) * 128], xts[sb][:, k, :],
                start=(k == 0), stop=(k == KCH - 1),
            )
        nc.vector.tensor_scalar_add(qkT[sb][:, m, :], u[:, 0:512], bqk[:, m : m + 1])

    def unit_norm(qb, hp, pvs2):
        qs = slice(qb * 512, (qb + 1) * 512)
        # evacuate the pv accumulator to SBUF fp16 right away so the single
        # 2-bank PSUM slot frees for the next head-pair (values are a few
        # thousand at most — far inside fp16 range).
        pvf = pf_p.tile([128, 1024], FP16, tag="pf", name="pvf")
        # two half-copies: each PSUM bank of the accumulator frees as soon
        # as its half is evacuated, so the next head-pair's first pv matmul
        # (same bank) unblocks ~0.5us earlier.
        for sub in range(2):
            nc.vector.tensor_copy(
                pvf[0:VW, sub * 512 : (sub + 1) * 512],
                pvs2[0:VW, sub * 512 : (sub + 1) * 512],
            )
        # reciprocal of the softmax denominator row, then rank-1 broadcast
        # to 64 partitions on the (otherwise idle) gpsimd engine.
        rse = rs_p.tile([1, 1024], F32R, tag="rs", name="rse")
        with nc.allow_low_precision(reason="f32r is full width"):
            nc.vector.reciprocal(rse[:], pvf[D : D + 1, :])
        bct = bc_p.tile([D, 1024], F32R, tag="bc", name="bct")
        nc.gpsimd.partition_broadcast(bct[:], rse[:])
        for sub in range(2):
            nc.vector.tensor_tensor(
                att[sub * 64 : sub * 64 + 64, hp, qs],
                pvf[0:D, sub * 512 : (sub + 1) * 512],
                bct[:, sub * 512 : (sub + 1) * 512],
                op=AluOpType.mult,
            )

    def unit_proj(qb, sc):
        sg = qb * 4 + sc
        ys = ys_p.tile([128, E], F32, tag="ys")
        for n0, nw in ((0, 512), (512, 256)):
            ya = st_p.tile([128, 1024], F32, tag="st", name="ya")
            for t in range(VC // 128):
                nc.tensor.matmul(
                    ya[:, :nw],
                    att[:, t, sg * 128 : (sg + 1) * 128],
                    wp[:, t, n0 : n0 + nw],
                    start=(t == 0), stop=(t == VC // 128 - 1),
                )
            nc.vector.tensor_copy(ys[:, n0 : n0 + nw], ya[:, :nw])
            # per-half store overlaps the DMA with the second half's matmuls
            nc.sync.dma_start(
                y_d.ap()[sg * 128 : (sg + 1) * 128, n0 : n0 + nw],
                ys[:, n0 : n0 + nw],
            )

    def emit_unit(u, state):
        kind = u[0]
        if kind == "tr":
            unit_tr(u[1])
        elif kind == "v":
            unit_v(u[1], u[2])
        elif kind == "qk":
            unit_qk(u[1], u[2])
        elif kind == "norm":
            unit_norm(u[1], u[2], state["pvs2"].pop((u[1], u[2])))
        elif kind == "proj":
            unit_proj(u[1], u[2])

    # prologue units: enough to start (qb0, hp0, kc0).
    for sg in range(4):
        unit_tr(sg)
    unit_qk(0, 0)
    unit_qk(0, 3)
    unit_v(0, 0)

    fillers = _build_schedule()
    state = {"pvs2": {}}
    pending = []  # (pt tile, qb, hp, kc) awaiting pv matmuls (depth-2 pipe)

    def emit_pv(prev):
        pt, qb, hp, kc = prev
        pvs2 = state["pvs2"][(qb, hp)]
        for sub in range(2):
            h = hp * 2 + sub
            nc.tensor.matmul(
                pvs2[0:VW, sub * 512 : (sub + 1) * 512],
                vst[kc // 4][:, kc % 4, h, :],
                pt[:, sub * 512 : (sub + 1) * 512],
                start=(kc == 0), stop=(kc == NKC - 1),
            )

    for i in range(NIT):
        kc = i % NKC
        hp = (i // NKC) % (HC // 2)
        qb = i // (NKC * (HC // 2))
        if kc == 0:
            state["pvs2"][(qb, hp)] = pv_p.tile(
                [128, 1024], F32, tag="pv", name=f"pv{qb}_{hp}"
            )
        # scores: both heads of the pair into one 2-bank f32 tile so one
        # exp instruction (free dim 1024) covers both.
        st = st_p.tile([128, 1024], F32, tag="st")
        for sub in range(2):
            kb, ko = kc // 4, kc % 4
            r0 = sub * 64
            nc.tensor.matmul(
                st[:, sub * 512 : (sub + 1) * 512],
                qkT[kb][r0 : r0 + 64, 3 + hp, ko * 128 : (ko + 1) * 128],
                qkT[qb][r0 : r0 + 64, hp, :],
                start=True, stop=True,
            )
        pt = pt_p.tile([128, 1024], FP16, tag="pt")
        nc.scalar.activation(pt[:], st[:], Act.Exp, scale=0.125)
        for u in fillers[i]:
            emit_unit(u, state)
        # depth-2 software pipeline; a head-pair's FIRST pv gets depth-3 so
        # it never waits on the pv-slot evacuation at the hp boundary.
        while pending:
            age = i - (
                (pending[0][1] * (HC // 2) + pending[0][2]) * NKC + pending[0][3]
            )
            need = 3 if pending[0][3] == 0 else 2
            if age < need:
                break
            emit_pv(pending.pop(0))
        pending.append((pt, qb, hp, kc))
    for p in pending:
        emit_pv(p)

    # tail: last head-pair's norm with the shortest possible chain — read
    # the pv accumulator in place and broadcast on PE (idle by now).
    pvs2 = state["pvs2"].pop((NQB - 1, HC // 2 - 1))
    qs = slice((NQB - 1) * 512, NQB * 512)
    # per-head reciprocal + gpsimd broadcast halves so each head's multiply
    # starts as soon as its own chain drains; the multiply reads the pv
    # accumulator in place (PSUM x SBUF — no evacuation copy needed).
    bct = bc_p.tile([D, 1024], F32R, tag="bc", name="bct_t")
    rse = rs_p.tile([1, 1024], F32R, tag="rs", name="rse_t")
    for sub in range(2):
        half = slice(sub * 512, (sub + 1) * 512)
        with nc.allow_low_precision(reason="f32r is full width"):
            nc.vector.reciprocal(rse[:, half], pvs2[D : D + 1, half])
        nc.gpsimd.partition_broadcast(bct[:, half], rse[:, half])
    # start the last q-block's projection on the heads that are already
    # normalized (t=0,1) while the reciprocal/broadcast chain drains; the
    # t=2 accumulation step joins after the final normalize below.
    yas = []
    for sc in range(3):
        sg = (NQB - 1) * 4 + sc
        ya = st_p.tile([128, 1024], F32, tag="st", name=f"yat{sc}")
        for n0, nw in ((0, 512), (512, 256)):
            for t in range(2):
                nc.tensor.matmul(
                    ya[:, n0 : n0 + nw],
                    att[:, t, sg * 128 : (sg + 1) * 128],
                    wp[:, t, n0 : n0 + nw],
                    start=(t == 0), stop=False,
                )
        yas.append(ya)
    for sub in range(2):
        nc.vector.tensor_tensor(
            att[sub * 64 : sub * 64 + 64, HC // 2 - 1, qs],
            pvs2[0:D, sub * 512 : (sub + 1) * 512],
            bct[:, sub * 512 : (sub + 1) * 512],
            op=AluOpType.mult,
        )
    for sc in range(3):
        sg = (NQB - 1) * 4 + sc
        ya = yas[sc]
        ys = ys_p.tile([128, E], F32, tag="ys")
        for n0, nw in ((0, 512), (512, 256)):
            nc.tensor.matmul(
                ya[:, n0 : n0 + nw],
                att[:, 2, sg * 128 : (sg + 1) * 128],
                wp[:, 2, n0 : n0 + nw],
                start=False, stop=True,
            )
            if n0 == 0:
                nc.vector.tensor_copy(ys[:, n0 : n0 + nw], ya[:, n0 : n0 + nw])
            else:
                nc.scalar.copy(ys[:, n0 : n0 + nw], ya[:, n0 : n0 + nw])
            nc.sync.dma_start(
                y_d.ap()[sg * 128 : (sg + 1) * 128, n0 : n0 + nw],
                ys[:, n0 : n0 + nw],
            )
    unit_proj(NQB - 1, 3)

    for p in reversed(ctx_pools):
        p.__exit__(None, None, None)


def make_core_inputs(x, mask, Wqkv, bqkv):
    """Slice full inputs into 8 per-core input maps."""
    x = np.asarray(x, dtype=np.float32)
    mask = np.asarray(mask)
    Wqkv = np.asarray(Wqkv, dtype=np.float32)
    bqkv = np.asarray(bqkv, dtype=np.float32)
    in_maps = []
    for c in range(8):
        b = c // 2
        h0 = (c % 2) * HC
        wq = Wqkv[:, h0 * D : (h0 + HC) * D]
        wk = Wqkv[:, E + h0 * D : E + (h0 + HC) * D]
        # [m, k, 128, 128]: per-m-chunk DMA granularity
        wqk = np.concatenate([wq, wk], axis=1).reshape(KCH, 128, QKC // 128, 128)
        wqk = wqk.transpose(2, 0, 1, 3)
        wv = Wqkv[:, 2 * E + h0 * D : 2 * E + (h0 + HC) * D].reshape(KCH, 128, VC)
        bqk = np.concatenate(
            [bqkv[h0 * D : (h0 + HC) * D], bqkv[E + h0 * D : E + (h0 + HC) * D]]
        )
        mv = (mask[b, 0, 0, :] != 0).astype(np.float32)  # 1.0 keep, 0.0 drop
        mb_t = mv.reshape(NKC, 128).T  # [p, c]
        mrep = np.repeat(mb_t[:, :, None], HC, axis=2)  # [p, c, h]
        in_maps.append(
            {
                "x": np.ascontiguousarray(x[b].astype(np.float16)),
                "wqk": np.ascontiguousarray(wqk.astype(np.float16)),
                "wv": np.ascontiguousarray(wv.astype(np.float16)),
                "wp": None,  # filled below (needs Wproj)
                "bqk": np.ascontiguousarray(
                    bqk.reshape(QKC // 128, 128).T.astype(np.float32).ravel()
                ),
                "mb": np.ascontiguousarray(mb_t.astype(np.float32).ravel()),
                "mrep": np.ascontiguousarray(mrep.astype(np.float16).ravel()),
            }
        )
    return in_maps


def run(x, mask, Wqkv, bqkv, Wproj, bproj, trace=False, trace_cores=None):
    Wproj = np.asarray(Wproj, dtype=np.float32)
    bproj = np.asarray(bproj, dtype=np.float32)
    bqkv_np = np.asarray(bqkv, dtype=np.float32)
    in_maps = make_core_inputs(x, mask, Wqkv, bqkv_np)
    for c in range(8):
        h0 = (c % 2) * HC
        wp = Wproj[h0 * D : (h0 + HC) * D, :].reshape(VC // 128, 128, E)
        in_maps[c]["wp"] = np.ascontiguousarray(wp.astype(np.float16))

    nc = build_program()
    try:
        res = run_bass_kernel_spmd(
            nc, in_maps, core_ids=list(range(8)), trace=trace,
            trace_cores=trace_cores,
        )
    except Exception:
        # transient device wedge (e.g. NRT_EXEC_UNIT_UNRECOVERABLE) —
        # one retry is usually enough
        res = run_bass_kernel_spmd(
            nc, in_maps, core_ids=list(range(8)), trace=trace,
            trace_cores=trace_cores,
        )
    parts = [res.results[c]["y"] for c in range(8)]

    # host-folded bias: v-bias passes through softmax (weights sum to 1),
    # so y += bv @ Wproj + bproj, applied once per batch row.
    bv = bqkv_np[2 * E : 3 * E]
    bias_row = bv @ Wproj + bproj
    y = np.stack(
        [parts[2 * b] + parts[2 * b + 1] + bias_row for b in range(B)]
    ).astype(np.float32)
    return y, res


def kernel(x, mask, Wqkv, bqkv, Wproj, bproj):
    y, _ = run(x, mask, Wqkv, bqkv, Wproj, bproj, trace=False)
    return y



# revision 4
# speedup vs baseline: 1.5963x; 1.5963x over previous
"""Trainium2 Bass kernel for CodeAttention (B=4, S=2048, E=768, H=12).

Sharding: 8 cores = 4 batches x 2 head-groups (6 heads each). Each core
computes a partial projection output for its batch; the host sums the two
partials per batch and adds the (host-folded) bias row.

Design (fp16 datapath, ~2x over the fused 270us baseline):
- Key compaction: the padding mask is known on the host, so masked keys
  (~50%) are gathered OUT of the K/V stream entirely (exact math: they
  contribute to neither the numerator nor the softmax denominator). Kept
  keys are padded to KP (multiple of 128) with zero columns whose ones-
  column entry is 0, which keeps them exactly inert.
- pv orientation flip: out[q,65] = sum_k pt[k,q]*vst[k,65] makes the
  moving operand the 65-wide V tile, cutting pv PE rows ~2x vs moving
  the 512-wide query block. The 65th column accumulates the softmax
  denominator, so normalization is a per-partition reciprocal+scale on
  DVE (no gpsimd broadcast).
- x arrives pre-transposed from the host (xt, xtk), so there are no
  on-chip x transposes; att is re-transposed on PE (48 small transposes)
  for the output projection, and y leaves as yT (host re-transposes).
- Main rhythm: per (query-half, head) 9 key-chunk slots, each = 2 score
  matmuls -> exp (ACT) -> one pv accumulation group of the previous head
  + statically scheduled filler units (q/k/v projections, attT, proj).
"""

import sys

if "/opt/trn_rl_repo" not in sys.path:
    sys.path.insert(0, "/opt/trn_rl_repo")

import numpy as np

import concourse.bass as bass  # noqa: F401
import concourse.mybir as mybir
import concourse.tile as tile
from concourse import bacc
from concourse.alu_op_type import AluOpType
from concourse.bass_utils import run_bass_kernel_spmd
from concourse.masks import make_identity

F32 = mybir.dt.float32
F32R = mybir.dt.float32r
FP16 = mybir.dt.float16
Act = mybir.ActivationFunctionType

B, S, E, H, D = 4, 2048, 768, 12, 64
HC = 6                    # heads per core
KCH = E // 128            # contraction chunks over E = 6
VC = HC * D               # v columns per core = 384
VW = D + 1                # v width incl. ones column = 65
DEFAULT_KP = 1152         # padded kept-key count for the fixed-seed mask


def build_program(kp=DEFAULT_KP):
    nkc = kp // 128
    nc = bacc.Bacc("TRN2", target_bir_lowering=False, debug=False, num_devices=8)

    xt_d = nc.dram_tensor("xt", [4, 128, KCH, 512], FP16, kind="ExternalInput")
    xtk_d = nc.dram_tensor("xtk", [128, KCH, kp], FP16, kind="ExternalInput")
    wq_d = nc.dram_tensor("wq", [128, KCH, 3, 128], FP16, kind="ExternalInput")
    wk_d = nc.dram_tensor("wk", [128, KCH, 3, 128], FP16, kind="ExternalInput")
    wv_d = nc.dram_tensor("wv", [128, KCH, VC], FP16, kind="ExternalInput")
    wp_d = nc.dram_tensor("wp", [128, 3, E], FP16, kind="ExternalInput")
    bq_d = nc.dram_tensor("bq", [128, 3], F32, kind="ExternalInput")
    bk_d = nc.dram_tensor("bk", [128, 3], F32, kind="ExternalInput")
    ones_d = nc.dram_tensor("ones", [128, nkc, HC], FP16, kind="ExternalInput")
    y_d = nc.dram_tensor("y", [HC, 128, S], FP16, kind="ExternalOutput")

    with tile.TileContext(nc) as tc:
        _emit(nc, tc, nkc, xt_d, xtk_d, wq_d, wk_d, wv_d, wp_d, bq_d, bk_d,
              ones_d, y_d)
    nc.compile()
    return nc


def _build_schedule(nkc):
    """slot -> list of filler units. Slots are (qbp, h, kc) flattened.

    Units: ("q", sb, m), ("k", m, kb), ("v", kc), ("at", qc8, dt, qbp),
    ("pj", Et, qb). Placement rules keep each unit >= a few slots ahead
    of its first consumer (see design notes in the module docstring).
    """
    fillers = {}

    def put(qbp, h, kc, u):
        i = (qbp * HC + h) * nkc + min(kc, nkc - 1)
        fillers.setdefault(i, []).append(u)

    # v units: vst[kc] needed when pv of head 0 runs (during h=1 slots)
    for kc in range(1, nkc):
        put(0, 0, kc - 1, ("v", kc))
    # q units (prologue does sb0/sb1 m0)
    put(0, 0, 3, ("q", 0, 1))
    put(0, 0, 5, ("q", 1, 1))
    put(0, 1, 3, ("q", 0, 2))
    put(0, 1, 5, ("q", 1, 2))
    put(0, 4, 2, ("q", 2, 0))
    put(0, 4, 4, ("q", 3, 0))
    put(1, 0, 2, ("q", 2, 1))
    put(1, 0, 4, ("q", 3, 1))
    put(1, 1, 2, ("q", 2, 2))
    put(1, 1, 4, ("q", 3, 2))
    # k units (prologue does m0); kb count = ceil(nkc/4)
    nkb = (nkc + 3) // 4
    for j in range(nkb):
        put(0, 0, 6 + 2 * j, ("k", 1, j))
    for j in range(nkb):
        put(0, 2, 6 + 2 * j, ("k", 2, j))
    # attT transposes: (qc8, dt) one head-period after norm(2dt+1, qc8)
    for qc8 in range(8):
        put(0, 3, qc8, ("at", qc8, 0, 0))
        put(0, 5, qc8, ("at", qc8, 1, 0))
        put(1, 1, qc8, ("at", qc8, 2, 0))
        put(1, 3, qc8, ("at", qc8, 0, 1))
        put(1, 5, qc8, ("at", qc8, 1, 1))
    # proj qb0/qb1 once attT dt2 for their q-chunks lands
    for Et in range(6):
        if Et < 3:
            put(1, 1, 5 + Et, ("pj", Et, 0))
        else:
            put(1, 2, Et - 3, ("pj", Et, 0))
        put(1, 2, 3 + Et, ("pj", Et, 1))
    return fillers


def _emit(nc, tc, nkc, xt_d, xtk_d, wq_d, wk_d, wv_d, wp_d, bq_d, bk_d,
          ones_d, y_d):
    kp = nkc * 128
    nkb = (kp + 511) // 512  # k-unit key blocks (512-wide, last ragged)
    ctx_pools = []

    def pool(name, bufs, space="SBUF"):
        p = tc.tile_pool(name=name, bufs=bufs, space=space)
        ctx_pools.append(p)
        return p.__enter__()

    consts = pool("consts", 1)
    store = pool("store", 1)
    pt_p = pool("pt", 2)
    sc_p = pool("sc", 2, space="PSUM")    # [128,1024] f32 = 2 banks each
    pv_p = pool("pv", 2, space="PSUM")    # [128,512] f32 = 1 bank each
    un_p = pool("un", 2, space="PSUM")    # [128,512] f32 = 1 bank each
    ys_p = pool("ys", 2)
    rs_p = pool("rs", 2)

    ident = consts.tile([128, 128], FP16)
    wq = consts.tile([128, KCH, 3, 128], FP16)
    wk = consts.tile([128, KCH, 3, 128], FP16)
    wv = consts.tile([128, KCH, VC], FP16)
    wp = consts.tile([128, 3, E], FP16)
    bq = consts.tile([128, 3], F32)
    bk = consts.tile([128, 3], F32)
    ones = consts.tile([128, nkc, HC], FP16)

    # weights on the SWDGE (gpsimd) queue, ordered by first use
    nc.gpsimd.dma_start(wq[:], wq_d.ap())
    make_identity(nc, ident[:])
    nc.gpsimd.dma_start(wk[:], wk_d.ap())
    nc.gpsimd.dma_start(wv[:], wv_d.ap())
    nc.gpsimd.dma_start(wp[:], wp_d.ap())
    # small tables on the scalar queue (done long before exps start)
    nc.scalar.dma_start(bq[:], bq_d.ap())
    nc.scalar.dma_start(bk[:], bk_d.ap())
    nc.scalar.dma_start(ones[:], ones_d.ap())

    xts = store.tile([128, KCH, S], FP16, name="xts")
    xtk = store.tile([128, KCH, kp], FP16, name="xtk")
    qT = [store.tile([128, 3, 512], FP16, name=f"qT{sb}") for sb in range(4)]
    kT = store.tile([128, 3, kp], FP16, name="kT")
    vst = store.tile([128, nkc, HC, VW], FP16, name="vst")
    att = store.tile([128, 16, VC], FP16, name="att")
    attT = store.tile([128, 3, S], FP16, name="attT")

    # x loads on the sync queue, halves first so q-unit matmuls can start
    # as soon as the first three contraction chunks land
    def load_xt(sb):
        for half in range(2):
            ks = slice(3 * half, 3 * half + 3)
            nc.sync.dma_start(
                xts[:, ks, sb * 512 : (sb + 1) * 512], xt_d.ap()[sb][:, ks, :]
            )

    def load_xtk(c0, c1):
        nc.sync.dma_start(xtk[:, :, c0:c1], xtk_d.ap()[:, :, c0:c1])

    load_xt(0)
    load_xt(1)
    load_xtk(0, 512)
    if kp > 512:
        load_xtk(512, min(kp, 1024))
    if kp > 1024:
        load_xtk(1024, kp)
    load_xt(2)
    load_xt(3)

    # ---- units -----------------------------------------------------------
    def unit_q(sb, m):
        u = un_p.tile([128, 512], F32, tag="un", name=f"uq{sb}_{m}")
        for k in range(KCH):
            nc.tensor.matmul(
                u[:], wq[:, k, m, :], xts[:, k, sb * 512 : (sb + 1) * 512],
                start=(k == 0), stop=(k == KCH - 1),
            )
        nc.vector.tensor_scalar_add(qT[sb][:, m, :], u[:], bq[:, m : m + 1])

    def unit_k(m, kb):
        c0, c1 = kb * 512, min((kb + 1) * 512, kp)
        u = un_p.tile([128, 512], F32, tag="un", name=f"uk{m}_{kb}")
        for k in range(KCH):
            nc.tensor.matmul(
                u[:, 0 : c1 - c0], wk[:, k, m, :], xtk[:, k, c0:c1],
                start=(k == 0), stop=(k == KCH - 1),
            )
        nc.vector.tensor_scalar_add(
            kT[:, m, c0:c1], u[:, 0 : c1 - c0], bk[:, m : m + 1]
        )

    def unit_v(kc):
        u = un_p.tile([128, 512], F32, tag="un", name=f"uv{kc}")
        for k in range(KCH):
            nc.tensor.matmul(
                u[:, 0:VC], xtk[:, k, kc * 128 : (kc + 1) * 128], wv[:, k, :],
                start=(k == 0), stop=(k == KCH - 1),
            )
        nc.vector.tensor_copy(
            vst[:, kc, :, 0:D], u[:, 0:VC].rearrange("p (h d) -> p h d", h=HC)
        )
        nc.vector.tensor_copy(
            vst[:, kc, :, D : D + 1],
            ones[:, kc : kc + 1, :].rearrange("p one h -> p h one"),
        )

    def unit_attT(qc8, dt, qbp):
        qc = qbp * 8 + qc8
        tr = un_p.tile([128, 128], FP16, tag="un", name=f"tr{qc}_{dt}")
        nc.tensor.matmul(
            tr[:], att[:, qc, dt * 128 : (dt + 1) * 128], ident[:],
            is_transpose=True, start=True, stop=True,
        )
        nc.vector.tensor_copy(attT[:, dt, qc * 128 : (qc + 1) * 128], tr[:])

    def unit_proj(Et, qb):
        u = un_p.tile([128, 512], F32, tag="un", name=f"up{Et}_{qb}")
        for dt in range(3):
            nc.tensor.matmul(
                u[:], wp[:, dt, Et * 128 : (Et + 1) * 128],
                attT[:, dt, qb * 512 : (qb + 1) * 512],
                start=(dt == 0), stop=(dt == 2),
            )
        ys = ys_p.tile([128, 512], FP16, tag="ys", name="ys")
        nc.vector.tensor_copy(ys[:], u[:])
        nc.sync.dma_start(y_d.ap()[Et][:, qb * 512 : (qb + 1) * 512], ys[:])

    def pv_group(pt, h, qc, qc8):
        acc = pv_p.tile([128, 512], F32, tag="pv", name=f"pv{qc}_{h}")
        for kc in range(nkc):
            nc.tensor.matmul(
                acc[:, 0:VW], pt[:, kc, qc8 * 128 : (qc8 + 1) * 128],
                vst[:, kc, h, :],
                start=(kc == 0), stop=(kc == nkc - 1),
            )
        rse = rs_p.tile([128, 1], F32, tag="rs", name="rse")
        with nc.allow_low_precision(reason="f32r is full width"):
            nc.vector.reciprocal(rse[:], acc[:, D : D + 1])
        nc.vector.tensor_scalar_mul(
            att[:, qc, h * D : (h + 1) * D], acc[:, 0:D], rse[:]
        )

    def emit_unit(u):
        kind = u[0]
        if kind == "q":
            unit_q(u[1], u[2])
        elif kind == "k":
            unit_k(u[1], u[2])
        elif kind == "v":
            unit_v(u[1])
        elif kind == "at":
            unit_attT(u[1], u[2], u[3])
        elif kind == "pj":
            unit_proj(u[1], u[2])

    # ---- prologue units --------------------------------------------------
    unit_q(0, 0)
    unit_q(1, 0)
    for kb in range(nkb):
        unit_k(0, kb)
    unit_v(0)

    fillers = _build_schedule(nkc)

    # ---- main loop -------------------------------------------------------
    pt_prev = None  # (pt tile, head, qbp) whose pv runs this head period
    pt_cur = None
    for qbp in range(2):
        for h in range(HC):
            hp, sub = h // 2, h % 2
            r0 = sub * 64
            pt_prev = pt_cur
            pt_cur = pt_p.tile([128, nkc, 1024], FP16, tag="pt",
                               name=f"pt{qbp}_{h}")
            for kc in range(nkc):
                i = (qbp * HC + h) * nkc + kc
                st = sc_p.tile([128, 1024], F32, tag="sc")
                for j in range(2):
                    sb = 2 * qbp + j
                    nc.tensor.matmul(
                        st[:, j * 512 : (j + 1) * 512],
                        kT[r0 : r0 + 64, hp, kc * 128 : (kc + 1) * 128],
                        qT[sb][r0 : r0 + 64, hp, :],
                        start=True, stop=True,
                    )
                nc.scalar.activation(pt_cur[:, kc, :], st[:], Act.Exp,
                                     scale=0.125)
                if pt_prev is not None:
                    if kc < min(8, nkc - 1):
                        pvs = [kc]
                    elif kc == nkc - 1:
                        pvs = list(range(min(8, nkc - 1), 8))
                    else:
                        pvs = []
                    ph = (h - 1) % HC
                    pqbp = qbp if h > 0 else qbp - 1
                    for qc8 in pvs:
                        pv_group(pt_prev, ph, pqbp * 8 + qc8, qc8)
                for u in fillers.get(i, ()):
                    emit_unit(u)

    # ---- tail: last head's pv + attT dt2 + proj qb2/qb3 ------------------
    for qc8 in range(8):
        pv_group(pt_cur, HC - 1, 8 + qc8, qc8)
        unit_attT(qc8, 2, 1)
        if qc8 == 3:
            for Et in range(6):
                unit_proj(Et, 2)
        if qc8 == 7:
            for Et in range(6):
                unit_proj(Et, 3)

    for p in reversed(ctx_pools):
        p.__exit__(None, None, None)


def make_core_inputs(x, mask, Wqkv, bqkv, Wproj, kp):
    """Slice full inputs into 8 per-core input maps (host-side layouts)."""
    x = np.asarray(x, np.float32)
    mask = np.asarray(mask)
    Wqkv = np.asarray(Wqkv, np.float32)
    bqkv = np.asarray(bqkv, np.float32)
    Wproj = np.asarray(Wproj, np.float32)
    nkc = kp // 128
    f16 = np.float16
    maps = []
    for c in range(8):
        b, hg = c // 2, c % 2
        h0 = hg * HC
        keep = np.nonzero(mask[b, 0, 0, :] != 0)[0]
        kept = len(keep)
        xt = x[b].T.reshape(KCH, 128, S).transpose(1, 0, 2)  # [p, kch, s]
        xt4 = np.ascontiguousarray(
            xt.reshape(128, KCH, 4, 512).transpose(2, 0, 1, 3).astype(f16)
        )
        xk = np.zeros((kp, E), np.float32)
        xk[:kept] = x[b, keep, :]
        xtk = np.ascontiguousarray(
            xk.T.reshape(KCH, 128, kp).transpose(1, 0, 2).astype(f16)
        )
        wq = Wqkv[:, h0 * D : (h0 + HC) * D]
        wq = np.ascontiguousarray(
            wq.reshape(KCH, 128, 3, 128).transpose(1, 0, 2, 3).astype(f16)
        )
        wkk = Wqkv[:, E + h0 * D : E + (h0 + HC) * D]
        wkk = np.ascontiguousarray(
            wkk.reshape(KCH, 128, 3, 128).transpose(1, 0, 2, 3).astype(f16)
        )
        wvv = Wqkv[:, 2 * E + h0 * D : 2 * E + (h0 + HC) * D]
        wvv = np.ascontiguousarray(
            wvv.reshape(KCH, 128, VC).transpose(1, 0, 2).astype(f16)
        )
        wpp = Wproj[hg * VC : (hg + 1) * VC, :]
        wpp = np.ascontiguousarray(
            wpp.reshape(3, 128, E).transpose(1, 0, 2).astype(f16)
        )
        bqq = np.ascontiguousarray(
            bqkv[h0 * D : (h0 + HC) * D].reshape(3, 128).T.astype(np.float32)
        )
        bkk = np.ascontiguousarray(
            bqkv[E + h0 * D : E + (h0 + HC) * D]
            .reshape(3, 128).T.astype(np.float32)
        )
        keepmask = (np.arange(kp) < kept).astype(f16).reshape(nkc, 128).T
        onesr = np.ascontiguousarray(
            np.repeat(keepmask[:, :, None], HC, axis=2).astype(f16)
        )
        maps.append(
            {
                "xt": xt4, "xtk": xtk, "wq": wq, "wk": wkk, "wv": wvv,
                "wp": wpp, "bq": bqq, "bk": bkk, "ones": onesr,
            }
        )
    return maps


def run(x, mask, Wqkv, bqkv, Wproj, bproj, trace=False, trace_cores=None):
    mask = np.asarray(mask)
    Wproj_np = np.asarray(Wproj, np.float32)
    bproj_np = np.asarray(bproj, np.float32)
    bqkv_np = np.asarray(bqkv, np.float32)
    kept = (mask[:, 0, 0, :] != 0).sum(axis=1)
    kp = max(128, int(-(-kept.max() // 128)) * 128)
    in_maps = make_core_inputs(x, mask, Wqkv, bqkv_np, Wproj_np, kp)

    nc = build_program(kp)
    try:
        res = run_bass_kernel_spmd(
            nc, in_maps, core_ids=list(range(8)), trace=trace,
            trace_cores=trace_cores,
        )
    except Exception:
        # transient device wedge -- one retry is usually enough
        res = run_bass_kernel_spmd(
            nc, in_maps, core_ids=list(range(8)), trace=trace,
            trace_cores=trace_cores,
        )

    # host-folded bias: v-bias passes through softmax (weights sum to 1)
    bv = bqkv_np[2 * E : 3 * E]
    bias_row = bv @ Wproj_np + bproj_np
    y = np.empty((B, S, E), np.float32)
    for b in range(B):
        p0 = res.results[2 * b]["y"].reshape(E, S).astype(np.float32)
        p1 = res.results[2 * b + 1]["y"].reshape(E, S).astype(np.float32)
        y[b] = p0.T + p1.T + bias_row
    return y, res


def kernel(x, mask, Wqkv, bqkv, Wproj, bproj):
    y, _ = run(x, mask, Wqkv, bqkv, Wproj, bproj, trace=False)
    return y


# revision 45
# speedup vs baseline: 1.7930x; 1.1232x over previous
"""Trainium2 Bass kernel for CodeAttention (B=4, S=2048, E=768, H=12).

Sharding: 8 cores = 4 batches x 2 head-groups (6 heads each). Each core
computes a partial projection output for its batch; the host sums the two
partials per batch and adds the (host-folded) bias row.

Design (fp16 datapath, ~2x over the fused 270us baseline):
- Key compaction: the padding mask is known on the host, so masked keys
  (~50%) are gathered OUT of the K/V stream entirely (exact math: they
  contribute to neither the numerator nor the softmax denominator). Kept
  keys are padded to KP (multiple of 128) with zero columns whose ones-
  column entry is 0, which keeps them exactly inert.
- pv orientation flip: out[q,65] = sum_k pt[k,q]*vst[k,65] makes the
  moving operand the 65-wide V tile, cutting pv PE rows ~2x vs moving
  the 512-wide query block. The 65th column accumulates the softmax
  denominator, so normalization is a per-partition reciprocal+scale on
  DVE (no gpsimd broadcast).
- x arrives pre-transposed from the host (xt, xtk), so there are no
  on-chip x transposes; att is re-transposed on PE (48 small transposes)
  for the output projection, and y leaves as yT (host re-transposes).
- Main rhythm: per (query-half, head) 9 key-chunk slots, each = 2 score
  matmuls -> exp (ACT) -> one pv accumulation group of the previous head
  + statically scheduled filler units (q/k/v projections, attT, proj).
"""

import sys

if "/opt/trn_rl_repo" not in sys.path:
    sys.path.insert(0, "/opt/trn_rl_repo")

import numpy as np

import concourse.bass as bass  # noqa: F401
import concourse.mybir as mybir
import concourse.tile as tile
from concourse import bacc
from concourse.alu_op_type import AluOpType
from concourse.bass_utils import run_bass_kernel_spmd
from concourse.masks import make_identity

F32 = mybir.dt.float32
F32R = mybir.dt.float32r
FP16 = mybir.dt.float16
Act = mybir.ActivationFunctionType

B, S, E, H, D = 4, 2048, 768, 12, 64
HC = 6                    # heads per core
KCH = E // 128            # contraction chunks over E = 6
VC = HC * D               # v columns per core = 384
VW = D + 1                # v width incl. ones column = 65
DEFAULT_KP = 1152         # padded kept-key count for the fixed-seed mask


def build_program(kp=DEFAULT_KP):
    nkc = kp // 128
    nc = bacc.Bacc("TRN2", target_bir_lowering=False, debug=False, num_devices=8)

    xt_d = nc.dram_tensor("xt", [4, 128, KCH, 512], FP16, kind="ExternalInput")
    xtk_d = nc.dram_tensor("xtk", [128, KCH, kp], FP16, kind="ExternalInput")
    wq_d = nc.dram_tensor("wq", [128, KCH, 3, 128], FP16, kind="ExternalInput")
    wk_d = nc.dram_tensor("wk", [128, KCH, 3, 128], FP16, kind="ExternalInput")
    wv_d = nc.dram_tensor("wv", [128, KCH, VC], FP16, kind="ExternalInput")
    wp_d = nc.dram_tensor("wp", [128, 3, E], FP16, kind="ExternalInput")
    bq_d = nc.dram_tensor("bq", [128, 3], F32, kind="ExternalInput")
    bk_d = nc.dram_tensor("bk", [128, 3], F32, kind="ExternalInput")
    ones_d = nc.dram_tensor("ones", [128, nkc, HC], FP16, kind="ExternalInput")
    y_d = nc.dram_tensor("y", [HC, 128, S], FP16, kind="ExternalOutput")

    with tile.TileContext(nc) as tc:
        _emit(nc, tc, nkc, xt_d, xtk_d, wq_d, wk_d, wv_d, wp_d, bq_d, bk_d,
              ones_d, y_d)
    nc.compile()
    return nc


def _build_schedule(nkc):
    """slot -> list of filler units. Slots are (qbp, h, kc) flattened.

    Units: ("q", sb, m), ("k", m, kb), ("v", kc), ("at", qc8, dt, qbp),
    ("pj", Et, qb). Placement rules keep each unit >= a few slots ahead
    of its first consumer (see design notes in the module docstring).
    """
    fillers = {}

    def put(qbp, h, kc, u):
        i = (qbp * HC + h) * nkc + min(kc, nkc - 1)
        fillers.setdefault(i, []).append(u)

    # v units, head-pair granular: pair p needed by pv(h=2p) which runs
    # during head 2p+1; spread them so no single head period overloads PE
    for kc in range(nkc):
        put(0, 0, kc, ("v", kc, 0))
        put(0, 2, kc, ("v", kc, 1))
        put(0, 4, kc, ("v", kc, 2))
    # k units (prologue does m0 kb0 only); m-tile m needed by heads 2m..;
    # kb block j only feeds score chunks kc >= 4j, so later blocks are JIT
    nkb = (nkc + 3) // 4
    for j in range(1, nkb):
        put(0, 0, 2 * j - 1, ("k", 0, j))
    put(0, 1, 0, ("k", 1, 0))
    put(0, 1, 6, ("k", 1, 1))
    put(0, 2, 1, ("k", 1, 2))
    put(0, 3, 5, ("k", 2, 0))
    put(0, 3, 7, ("k", 2, 1))
    put(0, 4, 1, ("k", 2, 2))
    # q units (prologue does sb0/sb1 m0); m-tile m needed by heads 2m
    put(0, 1, 2, ("q", 0, 1))
    put(0, 1, 4, ("q", 1, 1))
    put(0, 3, 1, ("q", 0, 2))
    put(0, 3, 3, ("q", 1, 2))
    put(0, 5, 1, ("q", 2, 0))
    put(0, 5, 3, ("q", 3, 0))
    put(1, 0, 1, ("q", 2, 1))
    put(1, 0, 3, ("q", 3, 1))
    put(1, 1, 1, ("q", 2, 2))
    put(1, 1, 3, ("q", 3, 2))
    # attT transposes: (qc8, dt) one head-period after norm(2dt+1, qc8)
    for qc8 in range(8):
        put(0, 3, qc8, ("at", qc8, 0, 0))
        put(0, 5, qc8, ("at", qc8, 1, 0))
        put(1, 1, qc8, ("at", qc8, 2, 0))
        put(1, 3, qc8, ("at", qc8, 0, 1))
        put(1, 5, qc8, ("at", qc8, 1, 1))
    # proj qb0/qb1 spread through the (light) qbp1 head periods
    for Et in range(6):
        if Et < 3:
            put(1, 1, 5 + Et, ("pj", Et, 0))
        else:
            put(1, 2, 2 * (Et - 3) + 1, ("pj", Et, 0))
        if Et < 4:
            put(1, 3, 2 * Et + 1, ("pj", Et, 1))
        else:
            put(1, 4, 2 * (Et - 4) + 1, ("pj", Et, 1))
    return fillers


def _emit(nc, tc, nkc, xt_d, xtk_d, wq_d, wk_d, wv_d, wp_d, bq_d, bk_d,
          ones_d, y_d):
    kp = nkc * 128
    nkb = (kp + 511) // 512  # k-unit key blocks (512-wide, last ragged)
    ctx_pools = []

    def pool(name, bufs, space="SBUF"):
        p = tc.tile_pool(name=name, bufs=bufs, space=space)
        ctx_pools.append(p)
        return p.__enter__()

    consts = pool("consts", 1)
    store = pool("store", 1)
    pt_p = pool("pt", 2)
    sc_p = pool("sc", 2, space="PSUM")    # [128,1024] f32 = 2 banks each
    pv_p = pool("pv", 2, space="PSUM")    # [128,512] f32 = 1 bank each
    un_p = pool("un", 2, space="PSUM")    # [128,512] f32 = 1 bank each
    ys_p = pool("ys", 4)
    rs_p = pool("rs", 2)

    ident = consts.tile([128, 128], FP16)
    wq = consts.tile([128, KCH, 3, 128], FP16)
    wk = consts.tile([128, KCH, 3, 128], FP16)
    wv = consts.tile([128, KCH, VC], FP16)
    wp = consts.tile([128, 3, E], FP16)
    bq = consts.tile([128, 3], F32)
    bk = consts.tile([128, 3], F32)
    ones = consts.tile([128, nkc, HC], FP16)

    # The modeled DMA bus is near serial and only per-queue FIFO order is
    # controllable (SWDGE desc-gen has no waits, so it races the bus), so
    # ALL input loads go on the sync queue in exact priority order: the
    # critical path to the first exp (wk m0, xtk c0, wq m0, xt sb0/sb1)
    # first, then everything else by first use.


    xts = store.tile([128, KCH, S], FP16, name="xts")
    xtk = store.tile([128, KCH, kp], FP16, name="xtk")
    qT = [store.tile([128, 3, 512], FP16, name=f"qT{sb}") for sb in range(4)]
    kT = store.tile([128, 3, kp], FP16, name="kT")
    vst = store.tile([128, nkc, HC, VW], FP16, name="vst")
    att = store.tile([128, 16, VC], FP16, name="att")
    attT = store.tile([128, 3, S], FP16, name="attT")

    # x loads on the sync queue, halves first so q-unit matmuls can start
    # as soon as the first three contraction chunks land
    def load_xt(sb):
        for half in range(2):
            ks = slice(3 * half, 3 * half + 3)
            nc.sync.dma_start(
                xts[:, ks, sb * 512 : (sb + 1) * 512], xt_d.ap()[sb][:, ks, :]
            )

    def load_xtk(c0, c1):
        nc.sync.dma_start(xtk[:, :, c0:c1], xtk_d.ap()[:, :, c0:c1])

    nc.sync.dma_start(wk[:, :, 0, :], wk_d.ap()[:, :, 0, :])
    # first key block in contraction-halves so the first k-unit matmuls
    # start one transfer earlier
    nc.sync.dma_start(xtk[:, 0:3, 0:512], xtk_d.ap()[:, 0:3, 0:512])
    nc.sync.dma_start(xtk[:, 3:6, 0:512], xtk_d.ap()[:, 3:6, 0:512])
    nc.sync.dma_start(bk[:], bk_d.ap())
    nc.sync.dma_start(wq[:, :, 0, :], wq_d.ap()[:, :, 0, :])
    nc.sync.dma_start(bq[:], bq_d.ap())
    load_xt(0)
    load_xt(1)
    nc.sync.dma_start(ones[:], ones_d.ap())
    nc.sync.dma_start(wv[:], wv_d.ap())
    make_identity(nc, ident[:])
    if kp > 512:
        load_xtk(512, min(kp, 1024))
    nc.sync.dma_start(wq[:, :, 1:3, :], wq_d.ap()[:, :, 1:3, :])
    nc.sync.dma_start(wk[:, :, 1:3, :], wk_d.ap()[:, :, 1:3, :])
    if kp > 1024:
        load_xtk(1024, kp)
    load_xt(2)
    nc.sync.dma_start(wp[:], wp_d.ap())
    load_xt(3)

    # ---- units -----------------------------------------------------------
    def unit_q(sb, m):
        u = un_p.tile([128, 512], F32, tag="un", name=f"uq{sb}_{m}")
        for k in range(KCH):
            nc.tensor.matmul(
                u[:], wq[:, k, m, :], xts[:, k, sb * 512 : (sb + 1) * 512],
                start=(k == 0), stop=(k == KCH - 1),
            )
        nc.vector.tensor_scalar_add(qT[sb][:, m, :], u[:], bq[:, m : m + 1])

    def unit_k(m, kb):
        c0, c1 = kb * 512, min((kb + 1) * 512, kp)
        u = un_p.tile([128, 512], F32, tag="un", name=f"uk{m}_{kb}")
        for k in range(KCH):
            nc.tensor.matmul(
                u[:, 0 : c1 - c0], wk[:, k, m, :], xtk[:, k, c0:c1],
                start=(k == 0), stop=(k == KCH - 1),
            )
        nc.vector.tensor_scalar_add(
            kT[:, m, c0:c1], u[:, 0 : c1 - c0], bk[:, m : m + 1]
        )

    def unit_v(kc, p):
        # one head-pair's v columns: keeps the v work out of the first
        # head period (pv of head h only needs pair h//2's columns)
        u = un_p.tile([128, 512], F32, tag="un", name=f"uv{kc}_{p}")
        for k in range(KCH):
            nc.tensor.matmul(
                u[:, 0:128], xtk[:, k, kc * 128 : (kc + 1) * 128],
                wv[:, k, p * 128 : (p + 1) * 128],
                start=(k == 0), stop=(k == KCH - 1),
            )
        nc.vector.tensor_copy(
            vst[:, kc, 2 * p : 2 * p + 2, 0:D],
            u[:, 0:128].rearrange("p (h d) -> p h d", h=2),
        )
        nc.vector.tensor_copy(
            vst[:, kc, 2 * p : 2 * p + 2, D : D + 1],
            ones[:, kc : kc + 1, 2 * p : 2 * p + 2].rearrange(
                "p one h -> p h one"
            ),
        )

    def unit_attT(qc8, dt, qbp, tail=False):
        qc = qbp * 8 + qc8
        tr = un_p.tile([128, 128], FP16, tag="un", name=f"tr{qc}_{dt}")
        nc.tensor.matmul(
            tr[:], att[:, qc, dt * 128 : (dt + 1) * 128], ident[:],
            is_transpose=True, start=True, stop=True,
        )
        # in the tail ACT is idle (exps done): evacuate there instead of
        # queueing behind the DVE norm chain
        if tail:
            nc.scalar.copy(attT[:, dt, qc * 128 : (qc + 1) * 128], tr[:])
        else:
            nc.vector.tensor_copy(attT[:, dt, qc * 128 : (qc + 1) * 128], tr[:])

    def unit_proj(Et, qb, tail=False):
        u = un_p.tile([128, 512], F32, tag="un", name=f"up{Et}_{qb}")
        for dt in range(3):
            nc.tensor.matmul(
                u[:], wp[:, dt, Et * 128 : (Et + 1) * 128],
                attT[:, dt, qb * 512 : (qb + 1) * 512],
                start=(dt == 0), stop=(dt == 2),
            )
        ys = ys_p.tile([128, 512], FP16, tag="ys", name="ys")
        if tail:
            nc.scalar.copy(ys[:], u[:])
        else:
            nc.vector.tensor_copy(ys[:], u[:])
        eng = nc.sync if (Et + qb) % 2 == 0 else nc.scalar
        eng.dma_start(y_d.ap()[Et][:, qb * 512 : (qb + 1) * 512], ys[:])

    def pv_group(pt, h, qcs):
        # one PSUM bank accumulates len(qcs) (<=2) query chunks: a single
        # start/stop accumulation group, halving pv ring turnover
        acc = pv_p.tile([128, 512], F32, tag="pv", name=f"pv{qcs[0]}_{h}")
        n = len(qcs)
        for kc in range(nkc):
            for x, qc in enumerate(qcs):
                nc.tensor.matmul(
                    acc[:, x * VW : (x + 1) * VW],
                    pt[:, kc, (qc % 8) * 128 : (qc % 8 + 1) * 128],
                    vst[:, kc, h, :],
                    start=(kc == 0 and x == 0),
                    stop=(kc == nkc - 1 and x == n - 1),
                )
        for x, qc in enumerate(qcs):
            rse = rs_p.tile([128, 1], F32, tag="rs", name="rse")
            with nc.allow_low_precision(reason="f32r is full width"):
                nc.vector.reciprocal(rse[:], acc[:, x * VW + D : x * VW + D + 1])
            nc.vector.tensor_scalar_mul(
                att[:, qc, h * D : (h + 1) * D],
                acc[:, x * VW : x * VW + D], rse[:],
            )

    def emit_unit(u):
        kind = u[0]
        if kind == "q":
            unit_q(u[1], u[2])
        elif kind == "k":
            unit_k(u[1], u[2])
        elif kind == "v":
            unit_v(u[1], u[2])
        elif kind == "at":
            unit_attT(u[1], u[2], u[3])
        elif kind == "pj":
            unit_proj(u[1], u[2])

    # ---- prologue units --------------------------------------------------
    unit_k(0, 0)
    unit_q(0, 0)

    fillers = _build_schedule(nkc)
    nslots = 2 * HC * nkc
    sc_pending = []  # score tiles awaiting their exp, FIFO

    def scores_mm(st, flat, j):
        qbp, rem = divmod(flat, HC * nkc)
        h, kc = divmod(rem, nkc)
        hp, r0 = h // 2, (h % 2) * 64
        sb = 2 * qbp + j
        nc.tensor.matmul(
            st[:, j * 512 : (j + 1) * 512],
            kT[r0 : r0 + 64, hp, kc * 128 : (kc + 1) * 128],
            qT[sb][r0 : r0 + 64, hp, :],
            start=True, stop=True,
        )

    def emit_scores(flat):
        st = sc_p.tile([128, 1024], F32, tag="sc", name="st")
        scores_mm(st, flat, 0)
        scores_mm(st, flat, 1)
        sc_pending.append(st)

    # front pipeline: the j0 halves of the first two score chunks depend
    # only on the sb0 query block (early on the DMA bus); q(1,0) and the
    # j1 halves follow once sb1 lands. Cross-engine deps are emission-
    # ordered, so this ordering is what lets ACT start ~2us earlier.
    if nkc >= 2:
        st0 = sc_p.tile([128, 1024], F32, tag="sc", name="st0")
        st1 = sc_p.tile([128, 1024], F32, tag="sc", name="st1")
        scores_mm(st0, 0, 0)
        scores_mm(st1, 1, 0)
    else:
        unit_q(1, 0)
        emit_scores(0)

    # ---- main loop: exp(i) -> scores(i+1) -> pv -> fillers ---------------
    pt_prev = None  # pt tile whose pv groups run during this head period
    pt_cur = None
    for qbp in range(2):
        for h in range(HC):
            pt_prev = pt_cur
            pt_cur = pt_p.tile([128, nkc, 1024], FP16, tag="pt",
                               name=f"pt{qbp}_{h}")
            for kc in range(nkc):
                i = (qbp * HC + h) * nkc + kc
                if i == 0 and nkc >= 2:
                    # first two slots: exp in 512-wide halves, j0 halves
                    # first -- the sb0 query block lands well before sb1 on
                    # the serialized DMA bus, so ACT starts ~4us earlier
                    for stx, kx in ((st0, 0), (st1, 1)):
                        nc.scalar.activation(
                            pt_cur[:, kx, 0:512], stx[:, 0:512], Act.Exp,
                            scale=0.125,
                        )
                    unit_q(1, 0)
                    scores_mm(st0, 0, 1)
                    scores_mm(st1, 1, 1)
                    emit_scores(2)
                    for stx, kx in ((st0, 0), (st1, 1)):
                        nc.scalar.activation(
                            pt_cur[:, kx, 512:1024], stx[:, 512:1024],
                            Act.Exp, scale=0.125,
                        )
                    for u in fillers.get(0, ()):
                        emit_unit(u)
                    continue
                if i == 1 and nkc >= 2:
                    for u in fillers.get(1, ()):
                        emit_unit(u)
                    continue
                st = sc_pending.pop(0)
                nc.scalar.activation(pt_cur[:, kc, :], st[:], Act.Exp,
                                     scale=0.125)
                if i + 1 < nslots:
                    emit_scores(i + 1)
                if pt_prev is not None:
                    if kc < min(4, nkc - 1):
                        pvs = [2 * kc, 2 * kc + 1]
                    elif kc == nkc - 1:
                        pvs = list(range(min(8, 2 * (nkc - 1)), 8))
                    else:
                        pvs = []
                    ph = (h - 1) % HC
                    pqbp = qbp if h > 0 else qbp - 1
                    for x in range(0, len(pvs), 2):
                        pv_group(pt_prev, ph,
                                 [pqbp * 8 + q for q in pvs[x : x + 2]])
                for u in fillers.get(i, ()):
                    emit_unit(u)

    # ---- tail: last head's pv + attT dt2 + proj qb2/qb3 ------------------
    # interleave so PE never sits on the pv->norm->attT DVE chains: each
    # attT lags its pv by one group, proj units weave between pv groups as
    # soon as their four attT columns are present.
    tail = []
    for pp in range(4):
        tail.append(("pv", pp))
        if pp >= 1:
            tail += [("at", 2 * pp - 2), ("at", 2 * pp - 1)]
        if pp >= 2:
            tail += [("pj", 3 * (pp - 2), 2), ("pj", 3 * (pp - 2) + 1, 2),
                     ("pj", 3 * (pp - 2) + 2, 2)]
    tail += [("at", 6), ("at", 7)]
    tail += [("pj", Et, 3) for Et in range(6)]
    for u in tail:
        if u[0] == "pv":
            pv_group(pt_cur, HC - 1, [8 + 2 * u[1], 9 + 2 * u[1]])
        elif u[0] == "at":
            unit_attT(u[1], 2, 1)
        else:
            unit_proj(u[1], u[2])

    for p in reversed(ctx_pools):
        p.__exit__(None, None, None)


def make_core_inputs(x, mask, Wqkv, bqkv, Wproj, kp):
    """Slice full inputs into 8 per-core input maps (host-side layouts)."""
    x = np.asarray(x, np.float32)
    mask = np.asarray(mask)
    Wqkv = np.asarray(Wqkv, np.float32)
    bqkv = np.asarray(bqkv, np.float32)
    Wproj = np.asarray(Wproj, np.float32)
    nkc = kp // 128
    f16 = np.float16
    maps = []
    for c in range(8):
        b, hg = c // 2, c % 2
        h0 = hg * HC
        keep = np.nonzero(mask[b, 0, 0, :] != 0)[0]
        kept = len(keep)
        xt = x[b].T.reshape(KCH, 128, S).transpose(1, 0, 2)  # [p, kch, s]
        xt4 = np.ascontiguousarray(
            xt.reshape(128, KCH, 4, 512).transpose(2, 0, 1, 3).astype(f16)
        )
        xk = np.zeros((kp, E), np.float32)
        xk[:kept] = x[b, keep, :]
        xtk = np.ascontiguousarray(
            xk.T.reshape(KCH, 128, kp).transpose(1, 0, 2).astype(f16)
        )
        wq = Wqkv[:, h0 * D : (h0 + HC) * D]
        wq = np.ascontiguousarray(
            wq.reshape(KCH, 128, 3, 128).transpose(1, 0, 2, 3).astype(f16)
        )
        wkk = Wqkv[:, E + h0 * D : E + (h0 + HC) * D]
        wkk = np.ascontiguousarray(
            wkk.reshape(KCH, 128, 3, 128).transpose(1, 0, 2, 3).astype(f16)
        )
        wvv = Wqkv[:, 2 * E + h0 * D : 2 * E + (h0 + HC) * D]
        wvv = np.ascontiguousarray(
            wvv.reshape(KCH, 128, VC).transpose(1, 0, 2).astype(f16)
        )
        wpp = Wproj[hg * VC : (hg + 1) * VC, :]
        wpp = np.ascontiguousarray(
            wpp.reshape(3, 128, E).transpose(1, 0, 2).astype(f16)
        )
        bqq = np.ascontiguousarray(
            bqkv[h0 * D : (h0 + HC) * D].reshape(3, 128).T.astype(np.float32)
        )
        bkk = np.ascontiguousarray(
            bqkv[E + h0 * D : E + (h0 + HC) * D]
            .reshape(3, 128).T.astype(np.float32)
        )
        keepmask = (np.arange(kp) < kept).astype(f16).reshape(nkc, 128).T
        onesr = np.ascontiguousarray(
            np.repeat(keepmask[:, :, None], HC, axis=2).astype(f16)
        )
        maps.append(
            {
                "xt": xt4, "xtk": xtk, "wq": wq, "wk": wkk, "wv": wvv,
                "wp": wpp, "bq": bqq, "bk": bkk, "ones": onesr,
            }
        )
    return maps


def run(x, mask, Wqkv, bqkv, Wproj, bproj, trace=False, trace_cores=None):
    mask = np.asarray(mask)
    Wproj_np = np.asarray(Wproj, np.float32)
    bproj_np = np.asarray(bproj, np.float32)
    bqkv_np = np.asarray(bqkv, np.float32)
    kept = (mask[:, 0, 0, :] != 0).sum(axis=1)
    kp = max(128, int(-(-kept.max() // 128)) * 128)
    in_maps = make_core_inputs(x, mask, Wqkv, bqkv_np, Wproj_np, kp)

    nc = build_program(kp)
    try:
        res = run_bass_kernel_spmd(
            nc, in_maps, core_ids=list(range(8)), trace=trace,
            trace_cores=trace_cores,
        )
    except Exception:
        # transient device wedge -- one retry is usually enough
        res = run_bass_kernel_spmd(
            nc, in_maps, core_ids=list(range(8)), trace=trace,
            trace_cores=trace_cores,
        )

    # host-folded bias: v-bias passes through softmax (weights sum to 1)
    bv = bqkv_np[2 * E : 3 * E]
    bias_row = bv @ Wproj_np + bproj_np
    y = np.empty((B, S, E), np.float32)
    for b in range(B):
        p0 = res.results[2 * b]["y"].reshape(E, S).astype(np.float32)
        p1 = res.results[2 * b + 1]["y"].reshape(E, S).astype(np.float32)
        y[b] = p0.T + p1.T + bias_row
    return y, res


def kernel(x, mask, Wqkv, bqkv, Wproj, bproj):
    y, _ = run(x, mask, Wqkv, bqkv, Wproj, bproj, trace=False)
    return y


# revision 54
# speedup vs baseline: 1.8145x; 1.0120x over previous
"""Trainium2 Bass kernel for CodeAttention (B=4, S=2048, E=768, H=12).

Sharding: 8 cores = 4 batches x 2 head-groups (6 heads each). Each core
computes a partial projection output for its batch; the host sums the two
partials per batch and adds the (host-folded) bias row.

Design (fp16 datapath, ~2x over the fused 270us baseline):
- Key compaction: the padding mask is known on the host, so masked keys
  (~50%) are gathered OUT of the K/V stream entirely (exact math: they
  contribute to neither the numerator nor the softmax denominator). Kept
  keys are padded to KP (multiple of 128) with zero columns whose ones-
  column entry is 0, which keeps them exactly inert.
- pv orientation flip: out[q,65] = sum_k pt[k,q]*vst[k,65] makes the
  moving operand the 65-wide V tile, cutting pv PE rows ~2x vs moving
  the 512-wide query block. The 65th column accumulates the softmax
  denominator, so normalization is a per-partition reciprocal+scale on
  DVE (no gpsimd broadcast).
- x arrives pre-transposed from the host (xt, xtk), so there are no
  on-chip x transposes; att is re-transposed on PE (48 small transposes)
  for the output projection, and y leaves as yT (host re-transposes).
- Main rhythm: per (query-half, head) 9 key-chunk slots, each = 2 score
  matmuls -> exp (ACT) -> one pv accumulation group of the previous head
  + statically scheduled filler units (q/k/v projections, attT, proj).
"""

import sys

if "/opt/trn_rl_repo" not in sys.path:
    sys.path.insert(0, "/opt/trn_rl_repo")

import numpy as np

import concourse.bass as bass  # noqa: F401
import concourse.mybir as mybir
import concourse.tile as tile
from concourse import bacc
from concourse.alu_op_type import AluOpType
from concourse.bass_utils import run_bass_kernel_spmd
from concourse.masks import make_identity

F32 = mybir.dt.float32
F32R = mybir.dt.float32r
FP16 = mybir.dt.float16
Act = mybir.ActivationFunctionType

B, S, E, H, D = 4, 2048, 768, 12, 64
HC = 6                    # heads per core
KCH = E // 128            # contraction chunks over E = 6
VC = HC * D               # v columns per core = 384
VW = D + 1                # v width incl. ones column = 65
DEFAULT_KP = 1152         # padded kept-key count for the fixed-seed mask


def build_program(kp=DEFAULT_KP):
    nkc = kp // 128
    nc = bacc.Bacc("TRN2", target_bir_lowering=False, debug=False, num_devices=8)

    xt_d = nc.dram_tensor("xt", [4, 128, KCH, 512], FP16, kind="ExternalInput")
    xtk_d = nc.dram_tensor("xtk", [128, KCH, kp], FP16, kind="ExternalInput")
    wq_d = nc.dram_tensor("wq", [128, KCH, 3, 128], FP16, kind="ExternalInput")
    wk_d = nc.dram_tensor("wk", [128, KCH, 3, 128], FP16, kind="ExternalInput")
    wv_d = nc.dram_tensor("wv", [128, KCH, VC], FP16, kind="ExternalInput")
    wp_d = nc.dram_tensor("wp", [128, 3, E], FP16, kind="ExternalInput")
    bq_d = nc.dram_tensor("bq", [128, 3], F32, kind="ExternalInput")
    bk_d = nc.dram_tensor("bk", [128, 3], F32, kind="ExternalInput")
    ones_d = nc.dram_tensor("ones", [128, nkc, HC], FP16, kind="ExternalInput")
    y_d = nc.dram_tensor("y", [HC, 128, S], FP16, kind="ExternalOutput")

    with tile.TileContext(nc) as tc:
        _emit(nc, tc, nkc, xt_d, xtk_d, wq_d, wk_d, wv_d, wp_d, bq_d, bk_d,
              ones_d, y_d)
    nc.compile()
    return nc


def _build_schedule(nkc):
    """slot -> list of filler units. Slots are (qbp, h, kc) flattened.

    Units: ("q", sb, m), ("k", m, kb), ("v", kc), ("at", qc8, dt, qbp),
    ("pj", Et, qb). Placement rules keep each unit >= a few slots ahead
    of its first consumer (see design notes in the module docstring).
    """
    fillers = {}

    def put(qbp, h, kc, u):
        i = (qbp * HC + h) * nkc + min(kc, nkc - 1)
        fillers.setdefault(i, []).append(u)

    # v units, head-pair granular: pair p needed by pv(h=2p) which runs
    # during head 2p+1; spread them so no single head period overloads PE
    for kc in range(nkc):
        put(0, 0, kc, ("v", kc, 0))
        put(0, 2, kc, ("v", kc, 1))
        put(0, 4, kc, ("v", kc, 2))
    # k units (prologue does m0 kb0 only); m-tile m needed by heads 2m..;
    # kb block j only feeds score chunks kc >= 4j, so later blocks are JIT
    nkb = (nkc + 3) // 4
    for j in range(1, nkb):
        put(0, 0, 2 * j - 1, ("k", 0, j))
    put(0, 1, 0, ("k", 1, 0))
    put(0, 1, 6, ("k", 1, 1))
    put(0, 2, 1, ("k", 1, 2))
    put(0, 3, 5, ("k", 2, 0))
    put(0, 3, 7, ("k", 2, 1))
    put(0, 4, 1, ("k", 2, 2))
    # q units (prologue does sb0/sb1 m0); m-tile m needed by heads 2m
    put(0, 1, 2, ("q", 0, 1))
    put(0, 1, 4, ("q", 1, 1))
    put(0, 3, 1, ("q", 0, 2))
    put(0, 3, 3, ("q", 1, 2))
    put(0, 5, 1, ("q", 2, 0))
    put(0, 5, 3, ("q", 3, 0))
    put(1, 0, 1, ("q", 2, 1))
    put(1, 0, 3, ("q", 3, 1))
    put(1, 1, 1, ("q", 2, 2))
    put(1, 1, 3, ("q", 3, 2))
    # attT transposes: (qc8, dt) one head-period after norm(2dt+1, qc8)
    for qc8 in range(8):
        put(0, 3, qc8, ("at", qc8, 0, 0))
        put(0, 5, qc8, ("at", qc8, 1, 0))
        put(1, 1, qc8, ("at", qc8, 2, 0))
        put(1, 3, qc8, ("at", qc8, 0, 1))
        put(1, 5, qc8, ("at", qc8, 1, 1))
    # proj qb0/qb1 spread through the (light) qbp1 head periods
    for Et in range(6):
        if Et < 3:
            put(1, 1, 5 + Et, ("pj", Et, 0))
        else:
            put(1, 2, 2 * (Et - 3) + 1, ("pj", Et, 0))
        if Et < 4:
            put(1, 3, 2 * Et + 1, ("pj", Et, 1))
        else:
            put(1, 4, 2 * (Et - 4) + 1, ("pj", Et, 1))
    return fillers


def _emit(nc, tc, nkc, xt_d, xtk_d, wq_d, wk_d, wv_d, wp_d, bq_d, bk_d,
          ones_d, y_d):
    kp = nkc * 128
    nkb = (kp + 511) // 512  # k-unit key blocks (512-wide, last ragged)
    ctx_pools = []

    def pool(name, bufs, space="SBUF"):
        p = tc.tile_pool(name=name, bufs=bufs, space=space)
        ctx_pools.append(p)
        return p.__enter__()

    consts = pool("consts", 1)
    store = pool("store", 1)
    pt_p = pool("pt", 2)
    sc_p = pool("sc", 2, space="PSUM")    # [128,1024] f32 = 2 banks each
    pv_p = pool("pv", 2, space="PSUM")    # [128,512] f32 = 1 bank each
    un_p = pool("un", 2, space="PSUM")    # [128,512] f32 = 1 bank each
    ys_p = pool("ys", 4)
    rs_p = pool("rs", 2)

    ident = consts.tile([128, 128], FP16)
    wq = consts.tile([128, KCH, 3, 128], FP16)
    wk = consts.tile([128, KCH, 3, 128], FP16)
    wv = consts.tile([128, KCH, VC], FP16)
    wp = consts.tile([128, 3, E], FP16)
    bq = consts.tile([128, 3], F32)
    bk = consts.tile([128, 3], F32)
    ones = consts.tile([128, nkc, HC], FP16)

    # The modeled DMA bus is near serial and only per-queue FIFO order is
    # controllable (SWDGE desc-gen has no waits, so it races the bus), so
    # ALL input loads go on the sync queue in exact priority order: the
    # critical path to the first exp (wk m0, xtk c0, wq m0, xt sb0/sb1)
    # first, then everything else by first use.


    xts = store.tile([128, KCH, S], FP16, name="xts")
    xtk = store.tile([128, KCH, kp], FP16, name="xtk")
    qT = [store.tile([128, 3, 512], FP16, name=f"qT{sb}") for sb in range(4)]
    kT = store.tile([128, 3, kp], FP16, name="kT")
    vst = store.tile([128, nkc, HC, VW], FP16, name="vst")
    att = store.tile([128, 16, VC], FP16, name="att")
    attT = store.tile([128, 3, S], FP16, name="attT")

    # x loads on the sync queue, halves first so q-unit matmuls can start
    # as soon as the first three contraction chunks land
    def load_xt(sb):
        for half in range(2):
            ks = slice(3 * half, 3 * half + 3)
            nc.sync.dma_start(
                xts[:, ks, sb * 512 : (sb + 1) * 512], xt_d.ap()[sb][:, ks, :]
            )

    def load_xtk(c0, c1):
        nc.sync.dma_start(xtk[:, :, c0:c1], xtk_d.ap()[:, :, c0:c1])

    nc.sync.dma_start(wk[:, :, 0, :], wk_d.ap()[:, :, 0, :])
    # first key block in contraction-halves so the first k-unit matmuls
    # start one transfer earlier
    nc.sync.dma_start(xtk[:, 0:3, 0:512], xtk_d.ap()[:, 0:3, 0:512])
    nc.sync.dma_start(xtk[:, 3:6, 0:512], xtk_d.ap()[:, 3:6, 0:512])
    nc.sync.dma_start(bk[:], bk_d.ap())
    nc.sync.dma_start(wq[:, :, 0, :], wq_d.ap()[:, :, 0, :])
    nc.sync.dma_start(bq[:], bq_d.ap())
    load_xt(0)
    load_xt(1)
    nc.sync.dma_start(ones[:], ones_d.ap())
    nc.sync.dma_start(wv[:], wv_d.ap())
    make_identity(nc, ident[:])
    if kp > 512:
        load_xtk(512, min(kp, 1024))
    nc.sync.dma_start(wq[:, :, 1:3, :], wq_d.ap()[:, :, 1:3, :])
    nc.sync.dma_start(wk[:, :, 1:3, :], wk_d.ap()[:, :, 1:3, :])
    if kp > 1024:
        load_xtk(1024, kp)
    load_xt(2)
    nc.sync.dma_start(wp[:], wp_d.ap())
    load_xt(3)

    # ---- units -----------------------------------------------------------
    def unit_q(sb, m):
        u = un_p.tile([128, 512], F32, tag="un", name=f"uq{sb}_{m}")
        for k in range(KCH):
            nc.tensor.matmul(
                u[:], wq[:, k, m, :], xts[:, k, sb * 512 : (sb + 1) * 512],
                start=(k == 0), stop=(k == KCH - 1),
            )
        nc.vector.tensor_scalar_add(qT[sb][:, m, :], u[:], bq[:, m : m + 1])

    def unit_k(m, kb):
        c0, c1 = kb * 512, min((kb + 1) * 512, kp)
        u = un_p.tile([128, 512], F32, tag="un", name=f"uk{m}_{kb}")
        for k in range(KCH):
            nc.tensor.matmul(
                u[:, 0 : c1 - c0], wk[:, k, m, :], xtk[:, k, c0:c1],
                start=(k == 0), stop=(k == KCH - 1),
            )
        nc.vector.tensor_scalar_add(
            kT[:, m, c0:c1], u[:, 0 : c1 - c0], bk[:, m : m + 1]
        )

    def unit_v(kc, p):
        # one head-pair's v columns: keeps the v work out of the first
        # head period (pv of head h only needs pair h//2's columns)
        u = un_p.tile([128, 512], F32, tag="un", name=f"uv{kc}_{p}")
        for k in range(KCH):
            nc.tensor.matmul(
                u[:, 0:128], xtk[:, k, kc * 128 : (kc + 1) * 128],
                wv[:, k, p * 128 : (p + 1) * 128],
                start=(k == 0), stop=(k == KCH - 1),
            )
        nc.vector.tensor_copy(
            vst[:, kc, 2 * p : 2 * p + 2, 0:D],
            u[:, 0:128].rearrange("p (h d) -> p h d", h=2),
        )
        nc.vector.tensor_copy(
            vst[:, kc, 2 * p : 2 * p + 2, D : D + 1],
            ones[:, kc : kc + 1, 2 * p : 2 * p + 2].rearrange(
                "p one h -> p h one"
            ),
        )

    def unit_attT(qc8, dt, qbp, pool=None):
        qc = qbp * 8 + qc8
        tr = (pool or un_p).tile([128, 128], FP16,
                                 tag="sc" if pool is sc_p else "un",
                                 name=f"tr{qc}_{dt}")
        nc.tensor.matmul(
            tr[:], att[:, qc, dt * 128 : (dt + 1) * 128], ident[:],
            is_transpose=True, start=True, stop=True,
        )
        nc.vector.tensor_copy(attT[:, dt, qc * 128 : (qc + 1) * 128], tr[:])

    def unit_proj(Et, qb, pool=None, evac=None):
        u = (pool or un_p).tile([128, 512], F32,
                                tag="pv" if pool is pv_p else "un",
                                name=f"up{Et}_{qb}")
        for dt in range(3):
            nc.tensor.matmul(
                u[:], wp[:, dt, Et * 128 : (Et + 1) * 128],
                attT[:, dt, qb * 512 : (qb + 1) * 512],
                start=(dt == 0), stop=(dt == 2),
            )
        ys = ys_p.tile([128, 512], FP16, tag="ys", name="ys")
        if evac is nc.scalar:
            nc.scalar.copy(ys[:], u[:])
            # keep the y-store config off the ACT SEQ (it would serialize
            # with the evacuation copies)
            nc.sync.dma_start(y_d.ap()[Et][:, qb * 512 : (qb + 1) * 512],
                              ys[:])
        else:
            nc.vector.tensor_copy(ys[:], u[:])
            eng = nc.sync if (Et + qb) % 2 == 0 else nc.scalar
            eng.dma_start(y_d.ap()[Et][:, qb * 512 : (qb + 1) * 512], ys[:])

    def pv_mms(acc, pt, h, qcs, kcs, start, stop):
        n = len(qcs)
        for ki, kc in enumerate(kcs):
            for x, qc in enumerate(qcs):
                nc.tensor.matmul(
                    acc[:, x * VW : (x + 1) * VW],
                    pt[:, kc, (qc % 8) * 128 : (qc % 8 + 1) * 128],
                    vst[:, kc, h, :],
                    start=(start and ki == 0 and x == 0),
                    stop=(stop and ki == len(kcs) - 1 and x == n - 1),
                )

    def pv_norms(acc, h, qcs):
        for x, qc in enumerate(qcs):
            rse = rs_p.tile([128, 1], F32, tag="rs", name="rse")
            with nc.allow_low_precision(reason="f32r is full width"):
                nc.vector.reciprocal(rse[:], acc[:, x * VW + D : x * VW + D + 1])
            nc.vector.tensor_scalar_mul(
                att[:, qc, h * D : (h + 1) * D],
                acc[:, x * VW : x * VW + D], rse[:],
            )

    def pv_group(pt, h, qcs):
        # one PSUM bank accumulates len(qcs) (<=2) query chunks: a single
        # start/stop accumulation group, halving pv ring turnover
        acc = pv_p.tile([128, 512], F32, tag="pv", name=f"pv{qcs[0]}_{h}")
        pv_mms(acc, pt, h, qcs, range(nkc), True, True)
        pv_norms(acc, h, qcs)

    def emit_unit(u):
        kind = u[0]
        if kind == "q":
            unit_q(u[1], u[2])
        elif kind == "k":
            unit_k(u[1], u[2])
        elif kind == "v":
            unit_v(u[1], u[2])
        elif kind == "at":
            unit_attT(u[1], u[2], u[3])
        elif kind == "pj":
            unit_proj(u[1], u[2])

    # ---- prologue units --------------------------------------------------
    unit_k(0, 0)
    unit_q(0, 0)

    fillers = _build_schedule(nkc)
    nslots = 2 * HC * nkc
    sc_pending = []  # score tiles awaiting their exp, FIFO

    def scores_mm(st, flat, j):
        qbp, rem = divmod(flat, HC * nkc)
        h, kc = divmod(rem, nkc)
        hp, r0 = h // 2, (h % 2) * 64
        sb = 2 * qbp + j
        nc.tensor.matmul(
            st[:, j * 512 : (j + 1) * 512],
            kT[r0 : r0 + 64, hp, kc * 128 : (kc + 1) * 128],
            qT[sb][r0 : r0 + 64, hp, :],
            start=True, stop=True,
        )

    def emit_scores(flat):
        st = sc_p.tile([128, 1024], F32, tag="sc", name="st")
        scores_mm(st, flat, 0)
        scores_mm(st, flat, 1)
        sc_pending.append(st)

    # front pipeline: the j0 halves of the first two score chunks depend
    # only on the sb0 query block (early on the DMA bus); q(1,0) and the
    # j1 halves follow once sb1 lands. Cross-engine deps are emission-
    # ordered, so this ordering is what lets ACT start ~2us earlier.
    if nkc >= 2:
        st0 = sc_p.tile([128, 1024], F32, tag="sc", name="st0")
        st1 = sc_p.tile([128, 1024], F32, tag="sc", name="st1")
        scores_mm(st0, 0, 0)
        scores_mm(st1, 1, 0)
    else:
        unit_q(1, 0)
        emit_scores(0)

    # ---- main loop: exp(i) -> scores(i+1) -> pv -> fillers ---------------
    pt_prev = None  # pt tile whose pv groups run during this head period
    pt_cur = None
    # last head: its pv pairs accumulate DURING its own slots ("tracking"),
    # using the idle un ring for pairs 0/1 and the pv ring (as its previous
    # user drains) for pairs 2/3 -- the tail then starts at norms directly
    track = [None] * 4
    talloc = {0: 0, 1: 0, 2: 4, 3: 5}  # pair -> first slot (catch-up there)
    for qbp in range(2):
        for h in range(HC):
            tracking = qbp == 1 and h == HC - 1 and nkc >= 6
            pt_prev = pt_cur
            pt_cur = pt_p.tile([128, nkc, 1024], FP16, tag="pt",
                               name=f"pt{qbp}_{h}")
            for kc in range(nkc):
                i = (qbp * HC + h) * nkc + kc
                if i == 0 and nkc >= 2:
                    # first two slots: exp in 512-wide halves, j0 halves
                    # first -- the sb0 query block lands well before sb1 on
                    # the serialized DMA bus, so ACT starts ~4us earlier
                    for stx, kx in ((st0, 0), (st1, 1)):
                        nc.scalar.activation(
                            pt_cur[:, kx, 0:512], stx[:, 0:512], Act.Exp,
                            scale=0.125,
                        )
                    unit_q(1, 0)
                    scores_mm(st0, 0, 1)
                    scores_mm(st1, 1, 1)
                    emit_scores(2)
                    for stx, kx in ((st0, 0), (st1, 1)):
                        nc.scalar.activation(
                            pt_cur[:, kx, 512:1024], stx[:, 512:1024],
                            Act.Exp, scale=0.125,
                        )
                    for u in fillers.get(0, ()):
                        emit_unit(u)
                    continue
                if i == 1 and nkc >= 2:
                    for u in fillers.get(1, ()):
                        emit_unit(u)
                    continue
                st = sc_pending.pop(0)
                nc.scalar.activation(pt_cur[:, kc, :], st[:], Act.Exp,
                                     scale=0.125)
                if i + 1 < nslots:
                    emit_scores(i + 1)
                if pt_prev is not None:
                    if kc < min(4, nkc - 1):
                        pvs = [2 * kc, 2 * kc + 1]
                    elif kc == nkc - 1:
                        pvs = list(range(min(8, 2 * (nkc - 1)), 8))
                    else:
                        pvs = []
                    ph = (h - 1) % HC
                    pqbp = qbp if h > 0 else qbp - 1
                    for x in range(0, len(pvs), 2):
                        pv_group(pt_prev, ph,
                                 [pqbp * 8 + q for q in pvs[x : x + 2]])
                if tracking:
                    for j in range(4):
                        k0 = talloc[j]
                        if kc < k0:
                            continue
                        qcs = [8 + 2 * j, 9 + 2 * j]
                        if kc == k0:
                            pl, tg = (un_p, "un") if j < 2 else (pv_p, "pv")
                            track[j] = pl.tile([128, 512], F32, tag=tg,
                                               name=f"tk{j}")
                            pv_mms(track[j], pt_cur, h, qcs,
                                   range(0, k0 + 1), True, kc == nkc - 1)
                        else:
                            pv_mms(track[j], pt_cur, h, qcs, [kc], False,
                                   kc == nkc - 1)
                for u in fillers.get(i, ()):
                    if tracking and u[0] == "at":
                        unit_attT(u[1], u[2], u[3], pool=sc_p)
                    else:
                        emit_unit(u)

    # ---- tail: last head's pv + attT dt2 + proj qb2/qb3 ------------------
    # interleave so PE never sits on the pv->norm->attT DVE chains: each
    # attT lags its pv by one group, proj units weave between pv groups as
    # soon as their four attT columns are present.
    if nkc >= 6:
        # tracked accumulators are complete right after the last exp: the
        # tail is just norms -> attT dt2 -> proj qb2/qb3, with the proj
        # units rotating over both freed PSUM rings (4 banks)
        pv_norms(track[0], HC - 1, [8, 9])
        pv_norms(track[1], HC - 1, [10, 11])
        for qc8 in range(4):
            unit_attT(qc8, 2, 1, pool=sc_p)
        pv_norms(track[2], HC - 1, [12, 13])
        pv_norms(track[3], HC - 1, [14, 15])
        ys2 = store.tile([128, HC, 512], FP16, name="ys2")

        def proj_qb2(Et, pl):
            u = pl.tile([128, 512], F32, tag="pv" if pl is pv_p else "un",
                        name=f"up{Et}_2")
            for dt in range(3):
                nc.tensor.matmul(
                    u[:], wp[:, dt, Et * 128 : (Et + 1) * 128],
                    attT[:, dt, 2 * 512 : 3 * 512],
                    start=(dt == 0), stop=(dt == 2),
                )
            if Et % 2 == 0:
                nc.scalar.copy(ys2[:, Et, :], u[:])
            else:
                nc.vector.tensor_copy(ys2[:, Et, :], u[:])
            if Et == 2:
                nc.sync.dma_start(
                    y_d.ap()[0:3, :, 2 * 512 : 3 * 512].rearrange(
                        "e p s -> p e s"
                    ),
                    ys2[:, 0:3, :],
                )
            elif Et == 5:
                nc.sync.dma_start(
                    y_d.ap()[3:6, :, 2 * 512 : 3 * 512].rearrange(
                        "e p s -> p e s"
                    ),
                    ys2[:, 3:6, :],
                )

        proj_qb2(0, un_p)
        proj_qb2(1, un_p)
        for qc8 in range(4, 8):
            unit_attT(qc8, 2, 1, pool=sc_p)
        for Et, pl in ((2, pv_p), (3, pv_p), (4, un_p), (5, un_p)):
            proj_qb2(Et, pl)
        # final query block: evacuate into one staging tile and store in
        # two 3-tile DMAs -- per-store HWDGE configs would serialize the
        # end-of-kernel chain
        ys6 = store.tile([128, HC, 512], FP16, name="ys6")
        for Et, pl in ((0, pv_p), (1, pv_p), (2, un_p), (3, un_p),
                       (4, pv_p), (5, pv_p)):
            u = pl.tile([128, 512], F32, tag="pv" if pl is pv_p else "un",
                        name=f"up{Et}_3")
            for dt in range(3):
                nc.tensor.matmul(
                    u[:], wp[:, dt, Et * 128 : (Et + 1) * 128],
                    attT[:, dt, 3 * 512 : 4 * 512],
                    start=(dt == 0), stop=(dt == 2),
                )
            if Et % 2 == 0:
                nc.scalar.copy(ys6[:, Et, :], u[:])
            else:
                nc.vector.tensor_copy(ys6[:, Et, :], u[:])
            if Et == 2:
                nc.sync.dma_start(
                    y_d.ap()[0:3, :, 3 * 512 : 4 * 512].rearrange(
                        "e p s -> p e s"
                    ),
                    ys6[:, 0:3, :],
                )
        nc.sync.dma_start(
            y_d.ap()[3:6, :, 3 * 512 : 4 * 512].rearrange("e p s -> p e s"),
            ys6[:, 3:6, :],
        )
    else:
        for pp in range(4):
            pv_group(pt_cur, HC - 1, [8 + 2 * pp, 9 + 2 * pp])
            if pp >= 1:
                unit_attT(2 * pp - 2, 2, 1)
                unit_attT(2 * pp - 1, 2, 1)
        unit_attT(6, 2, 1)
        unit_attT(7, 2, 1)
        for Et in range(6):
            unit_proj(Et, 2)
        for Et in range(6):
            unit_proj(Et, 3)

    for p in reversed(ctx_pools):
        p.__exit__(None, None, None)


def make_core_inputs(x, mask, Wqkv, bqkv, Wproj, kp):
    """Slice full inputs into 8 per-core input maps (host-side layouts)."""
    x = np.asarray(x, np.float32)
    mask = np.asarray(mask)
    Wqkv = np.asarray(Wqkv, np.float32)
    bqkv = np.asarray(bqkv, np.float32)
    Wproj = np.asarray(Wproj, np.float32)
    nkc = kp // 128
    f16 = np.float16
    maps = []
    for c in range(8):
        b, hg = c // 2, c % 2
        h0 = hg * HC
        keep = np.nonzero(mask[b, 0, 0, :] != 0)[0]
        kept = len(keep)
        xt = x[b].T.reshape(KCH, 128, S).transpose(1, 0, 2)  # [p, kch, s]
        xt4 = np.ascontiguousarray(
            xt.reshape(128, KCH, 4, 512).transpose(2, 0, 1, 3).astype(f16)
        )
        xk = np.zeros((kp, E), np.float32)
        xk[:kept] = x[b, keep, :]
        xtk = np.ascontiguousarray(
            xk.T.reshape(KCH, 128, kp).transpose(1, 0, 2).astype(f16)
        )
        wq = Wqkv[:, h0 * D : (h0 + HC) * D]
        wq = np.ascontiguousarray(
            wq.reshape(KCH, 128, 3, 128).transpose(1, 0, 2, 3).astype(f16)
        )
        wkk = Wqkv[:, E + h0 * D : E + (h0 + HC) * D]
        wkk = np.ascontiguousarray(
            wkk.reshape(KCH, 128, 3, 128).transpose(1, 0, 2, 3).astype(f16)
        )
        wvv = Wqkv[:, 2 * E + h0 * D : 2 * E + (h0 + HC) * D]
        wvv = np.ascontiguousarray(
            wvv.reshape(KCH, 128, VC).transpose(1, 0, 2).astype(f16)
        )
        wpp = Wproj[hg * VC : (hg + 1) * VC, :]
        wpp = np.ascontiguousarray(
            wpp.reshape(3, 128, E).transpose(1, 0, 2).astype(f16)
        )
        bqq = np.ascontiguousarray(
            bqkv[h0 * D : (h0 + HC) * D].reshape(3, 128).T.astype(np.float32)
        )
        bkk = np.ascontiguousarray(
            bqkv[E + h0 * D : E + (h0 + HC) * D]
            .reshape(3, 128).T.astype(np.float32)
        )
        keepmask = (np.arange(kp) < kept).astype(f16).reshape(nkc, 128).T
        onesr = np.ascontiguousarray(
            np.repeat(keepmask[:, :, None], HC, axis=2).astype(f16)
        )
        maps.append(
            {
                "xt": xt4, "xtk": xtk, "wq": wq, "wk": wkk, "wv": wvv,
                "wp": wpp, "bq": bqq, "bk": bkk, "ones": onesr,
            }
        )
    return maps


def run(x, mask, Wqkv, bqkv, Wproj, bproj, trace=False, trace_cores=None):
    mask = np.asarray(mask)
    Wproj_np = np.asarray(Wproj, np.float32)
    bproj_np = np.asarray(bproj, np.float32)
    bqkv_np = np.asarray(bqkv, np.float32)
    kept = (mask[:, 0, 0, :] != 0).sum(axis=1)
    kp = max(128, int(-(-kept.max() // 128)) * 128)
    in_maps = make_core_inputs(x, mask, Wqkv, bqkv_np, Wproj_np, kp)

    nc = build_program(kp)
    try:
        res = run_bass_kernel_spmd(
            nc, in_maps, core_ids=list(range(8)), trace=trace,
            trace_cores=trace_cores,
        )
    except Exception:
        # transient device wedge -- one retry is usually enough
        res = run_bass_kernel_spmd(
            nc, in_maps, core_ids=list(range(8)), trace=trace,
            trace_cores=trace_cores,
        )

    # host-folded bias: v-bias passes through softmax (weights sum to 1)
    bv = bqkv_np[2 * E : 3 * E]
    bias_row = bv @ Wproj_np + bproj_np
    y = np.empty((B, S, E), np.float32)
    for b in range(B):
        p0 = res.results[2 * b]["y"].reshape(E, S).astype(np.float32)
        p1 = res.results[2 * b + 1]["y"].reshape(E, S).astype(np.float32)
        y[b] = p0.T + p1.T + bias_row
    return y, res


def kernel(x, mask, Wqkv, bqkv, Wproj, bproj):
    y, _ = run(x, mask, Wqkv, bqkv, Wproj, bproj, trace=False)
    return y
